# revision 10
# baseline (speedup 1.0000x reference)
"""Causal self-attention (B=2, S=2048, D=1024, H=16) on 8 NeuronCores, v2.

Sharding: data-parallel over batch (2 groups of 4 cores), tensor-parallel
over heads within a group (4 heads / core). Host sums the 4 partial
[2048, 1024] out-projection results per batch.

v2 redesign vs v1 (cost-model driven):
  - All matmul operands bf16 (same PE rate as fp32r in-model, halves DMA
    and SBUF, rel err ~3e-3 vs the 2e-2 gate).
  - PV is query-major: P tiles for a whole q-chunk persist in SBUF, then
    per (head, 128-q-tile) bursts accumulate out [128 q, 65] with P as
    the stationary operand, moving only 65 V-columns (35k PE rows vs
    82k), full 128 out partitions, and the softmax denominator (ones
    column of V') lands per-partition so normalize is a cheap DVE
    reciprocal + per-partition broadcast multiply (no more K=1 broadcast
    matmuls). PSUM zero-region rule (one accumulation group per 2KB
    bank) is satisfied by ping-ponging two 1-bank accumulators.
  - Attention output returns channel-major via tiny PE transposes
    (bf16, 128 rows each) feeding the out-projection per q-tile.
  - Software-pipelined emission: exp(kt) on Act overlaps later scores /
    projection chunks / out-projection work injected from a filler
    queue, so PE never sits behind Act.
  - x/weights DMA'd in bf16, x by 512-column chunks so projections start
    after ~5us instead of 25us; y written bf16 (host upcasts + sums).
"""

import os
import numpy as np
import ml_dtypes

import concourse.bass as bass
import concourse.mybir as mybir
import concourse.tile as tile
from concourse.bass_utils import run_bass_kernel_spmd

F32 = mybir.dt.float32
BF16 = mybir.dt.bfloat16
FP8 = mybir.dt.float8e4
AF = mybir.ActivationFunctionType
DROW = mybir.MatmulPerfMode.DoubleRow

KT_BUDGET = float(os.environ.get("KT_BUDGET", 800))
J_BUDGET = float(os.environ.get("J_BUDGET", 400))
OP_EST = float(os.environ.get("OP_EST", 1500))
OP_CAP2 = float(os.environ.get("OP_CAP2", 3500))

BR_KT = int(os.environ.get("BR_KT", 4))

B, S, D, H = 2, 2048, 1024, 16
DH = D // H              # 64
HL = 4                   # heads per core
CL = HL * DH             # 256 channels per core
G = 4                    # cores per batch group
SCALE = DH ** -0.5       # 0.125
NQC = S // 512           # 4 q-chunks of 512
NKT = S // 128           # 16 key tiles of 128


def _split_excess_waits(nc, max_waits=1):
    """walrus in this toolchain rejects instructions carrying more than
    `max_waits` sem waits; split the excess onto preceding same-engine
    NoOps (sound: waits are monotone >= conditions hoisted earlier on
    the same engine)."""
    n_split = 0
    for f in nc.m.functions:
        for bb in f.blocks:
            out = []
            for inst in bb.instructions:
                si = inst.sync_info
                waits = list(si.on_wait) if si is not None and si.on_wait else []
                if len(waits) > max_waits:
                    head, keep = waits[:-max_waits], waits[-max_waits:]
                    for ci, start in enumerate(range(0, len(head), max_waits)):
                        nop = mybir.InstNoOp(
                            name=f"{inst.name}_wsplit{ci}",
                            sync_info=mybir.SyncInfo(
                                on_wait=head[start:start + max_waits],
                                on_update=[],
                            ),
                            engine=inst.engine,
                            bass_nofuse=True,
                        )
                        out.append(nop)
                        n_split += 1
                    si.on_wait = keep
                out.append(inst)
            if n_split:
                bb.instructions.clear()
                for i in out:
                    bb.instructions.append(i)
    return n_split


def _build_nc(split_waits=True):
    nc = bass.Bass()
    xt_d = nc.dram_tensor("xt", [D, S], BF16, kind="ExternalInput")
    wq_d = nc.dram_tensor("wq", [D, CL], BF16, kind="ExternalInput")
    wk_d = nc.dram_tensor("wk", [D, CL], BF16, kind="ExternalInput")
    wv_d = nc.dram_tensor("wv", [D, CL], BF16, kind="ExternalInput")
    wo_d = nc.dram_tensor("wo", [128, 2, D], BF16, kind="ExternalInput")
    mask_d = nc.dram_tensor("mask", [128, 2, 128], BF16, kind="ExternalInput")
    ident_d = nc.dram_tensor("ident", [128, 128], BF16, kind="ExternalInput")
    y_d = nc.dram_tensor("y", [S, D], BF16, kind="ExternalOutput")

    xt_r = xt_d.rearrange("(a p) s -> p a s", p=128)       # [128, 8, S]
    y_r = y_d.rearrange("(t p) (n d) -> p t n d", p=128, n=2)  # [128,16,2,512]

    with tile.TileContext(nc) as tc:
        with (
            tc.tile_pool(name="persist", bufs=1) as pp,
            tc.tile_pool(name="psum_st", bufs=2, space="PSUM") as stp,
            tc.tile_pool(name="psum_op", bufs=2, space="PSUM") as opp,
            tc.tile_pool(name="psum_sm", bufs=2, space="PSUM") as smp,
            tc.tile_pool(name="ptw", bufs=1) as ptp,
            tc.tile_pool(name="brdg", bufs=2) as brp,
            tc.tile_pool(name="onrm", bufs=2) as onp,
            tc.tile_pool(name="ri", bufs=4) as rip,
            tc.tile_pool(name="os", bufs=32) as osp,
            tc.tile_pool(name="ysb", bufs=8) as ysp,
        ):
            # ---- persistent SBUF ----------------------------------------
            xt_sb = pp.tile([128, 8, S], BF16)
            wq_sb = pp.tile([128, 8, CL], BF16)
            wk_sb = pp.tile([128, 8, CL], BF16)
            wv_sb = pp.tile([128, 8, CL], BF16)
            wo_sb = pp.tile([128, 2, D], BF16)
            mask_sb = pp.tile([128, 2, 128], BF16)
            ident_sb = pp.tile([128, 128], BF16)
            qt_sb = [pp.tile([128, S], BF16, name=f"qt{p}", tag=f"qt{p}")
                     for p in range(2)]
            kt_sb = [pp.tile([128, S], BF16, name=f"kt{p}", tag=f"kt{p}")
                     for p in range(2)]
            # V' per key tile: 4 heads x [64 v-cols + 1 ones-col]
            vp_sb = pp.tile([128, NKT, 4 * 65], BF16)

            for hl in range(4):
                nc.vector.memset(vp_sb[:, :, hl * 65 + 64:hl * 65 + 65], 1.0)

            # ---- input DMAs (SP queue), ordered for earliest start ------
            wq_r = wq_d.rearrange("(a p) m -> p a m", p=128)
            wk_r = wk_d.rearrange("(a p) m -> p a m", p=128)
            wv_r = wv_d.rearrange("(a p) m -> p a m", p=128)
            nc.sync.dma_start(wq_sb[:, 0:4, :], wq_r[:, 0:4, :])
            nc.sync.dma_start(xt_sb[:, :, 0:256], xt_r[:, :, 0:256])
            nc.sync.dma_start(wq_sb[:, 4:8, :], wq_r[:, 4:8, :])
            nc.sync.dma_start(wk_sb[:], wk_r)
            nc.sync.dma_start(wv_sb[:], wv_r)
            nc.sync.dma_start(xt_sb[:, :, 256:512], xt_r[:, :, 256:512])
            nc.sync.dma_start(xt_sb[:, :, 512:1024], xt_r[:, :, 512:1024])
            nc.sync.dma_start(mask_sb[:], mask_d[:, :, :])
            nc.sync.dma_start(ident_sb[:], ident_d[:, :])
            nc.sync.dma_start(xt_sb[:, :, 1024:1536], xt_r[:, :, 1024:1536])
            nc.sync.dma_start(wo_sb[:], wo_d[:, :, :])
            nc.sync.dma_start(xt_sb[:, :, 1536:2048], xt_r[:, :, 1536:2048])

            # ---- filler machinery ---------------------------------------
            # FIFOs of closures emitting deferrable PE work, injected
            # between attention units so PE never waits on Act's exp.
            # The Act exp stream is heavily back-loaded (the q-chunk qc
            # covers 4(qc+1) key tiles), so work is pushed as late as its
            # data dependencies allow:
            #   fill_late[c]: K/V projections of chunk c — only needed by
            #     q-chunk c itself from key-tile 4c on, so they run inside
            #     qc=c's own kt-loop (hard-drained at kt=4c).
            #   fill_proj: Q projection of chunk c+1 — needed at the next
            #     q-chunk's first score (drained at its loop start).
            #   fill_op: out-projections of finished q-chunks — kept for
            #     the starved qc>=2 loops (qc2 capped so qc3 gets most).
            fill_proj = []
            fill_late = []
            fill_op = []
            late_kv = {}
            op_state = {"spent": 0.0, "cap": 0.0, "qc": 0}

            def emit_filler(max_ns):
                spent = 0.0
                while spent < max_ns:
                    if fill_late:
                        _, est, fn = fill_late.pop(0)
                    elif fill_proj and fill_proj[0][0] <= op_state["qc"] + 2:
                        _, est, fn = fill_proj.pop(0)
                    elif fill_op and op_state["spent"] < op_state["cap"]:
                        est, fn = fill_op.pop(0)
                        op_state["spent"] += est
                    else:
                        break
                    fn()
                    spent += est

            def drain_proj(chunk):
                while fill_proj and fill_proj[0][0] <= chunk:
                    fill_proj.pop(0)[2]()

            def drain_late(chunk):
                while fill_late and fill_late[0][0] <= chunk:
                    fill_late.pop(0)[2]()

            def drain_all():
                drain_late(NQC)
                while fill_proj:
                    fill_proj.pop(0)[2]()
                while fill_op:
                    fill_op.pop(0)[1]()

            # ---- projections --------------------------------------------
            _proj_ps = {}

            def proj_qk(c, p, which, klo, half=None):
                w_sb, dst = ((wq_sb, qt_sb) if which == "q" else
                             (wk_sb, kt_sb))
                if half is None:
                    cslc = slice(c * 512, (c + 1) * 512)
                else:
                    cslc = slice(c * 512 + half * 256,
                                 c * 512 + (half + 1) * 256)
                pslc = slice(p * 128, (p + 1) * 128)
                key = (c, p, which, half)
                if klo == 0:
                    _proj_ps[key] = smp.tile([128, 512], F32, name="psqk",
                                             tag="sm")
                ps = _proj_ps[key]
                w = cslc.stop - cslc.start
                for k in range(klo, klo + 4):
                    nc.tensor.matmul(ps[:, 0:w], w_sb[:, k, pslc],
                                     xt_sb[:, k, cslc],
                                     start=(k == 0), stop=(k == 7))
                if klo == 4:
                    nc.vector.tensor_copy(dst[p][:, cslc], ps[:, 0:w])
                    del _proj_ps[key]

            def proj_v(st, klo):
                key = ("v", st)
                if klo == 0:
                    _proj_ps[key] = smp.tile([128, CL], F32, name="psv",
                                             tag="sm")
                ps = _proj_ps[key]
                for k in range(klo, klo + 4):
                    nc.tensor.matmul(ps[:],
                                     xt_sb[:, k, st * 128:(st + 1) * 128],
                                     wv_sb[:, k, :], start=(k == 0),
                                     stop=(k == 7))
                if klo == 4:
                    nc.vector.tensor_copy(
                        vp_sb[:, st, :]
                        .rearrange("p (h e) -> p h e", e=65)[:, :, 0:64],
                        ps[:].rearrange("p (h d) -> p h d", d=64))
                    del _proj_ps[key]

            def push_proj_chunks():
                for c in range(1, NQC):
                    for p in range(2):
                        for klo in (0, 4):
                            fill_proj.append((c, 900, (
                                lambda c=c, p=p, kl=klo:
                                proj_qk(c, p, "q", kl))))
                    kv = late_kv.setdefault(c, [])
                    for p in range(2):
                        for klo in (0, 4):
                            kv.append((c, 900, (
                                lambda c=c, p=p, kl=klo:
                                proj_qk(c, p, "k", kl))))
                    for st in range(4 * c, 4 * (c + 1)):
                        for klo in (0, 4):
                            kv.append((c, 470, (
                                lambda st=st, kl=klo: proj_v(st, kl))))

            # ---- attention pieces ---------------------------------------
            def scores(qc, p, kt, PTW):
                qlo = qc * 512
                dq = max(0, kt * 128 - qlo)
                ST = stp.tile([128, 2, 512], F32, name="ST", tag="st")
                for hi in range(2):
                    hslc = slice(hi * 64, (hi + 1) * 64)
                    nc.tensor.matmul(
                        ST[:, hi, dq:],
                        kt_sb[p][hslc, kt * 128:(kt + 1) * 128],
                        qt_sb[p][hslc, qlo + dq:qlo + 512],
                        start=True, stop=True)
                nc.scalar.activation(PTW[:, kt, :, dq:], ST[:, :, dq:],
                                     AF.Exp, scale=SCALE)
                if dq or kt * 128 == qlo:   # diagonal tile
                    nc.gpsimd.tensor_mul(
                        PTW[:, kt, :, dq:dq + 128],
                        PTW[:, kt, :, dq:dq + 128], mask_sb[:])

            def pv_burst(qc, p, hi, j, PTW, O_sb_p, bridge=None):
                bc = (2 * p + hi) * 65
                jslc = slice(j * 128, (j + 1) * 128)
                last = 4 * qc + j
                OPu = opp.tile([128, 65], F32, name="OPu", tag="op")
                for kt in range(last + 1):
                    src_pt = (bridge if bridge is not None and kt < BR_KT
                              else PTW)
                    nc.tensor.matmul(OPu[:], src_pt[:, kt, hi, jslc],
                                     vp_sb[:, kt, bc:bc + 65],
                                     start=(kt == 0), stop=(kt == last))
                Ri = rip.tile([128, 1], F32, name="Ri", tag="ri")
                nc.vector.reciprocal(Ri[:], OPu[:, 64:65])
                nc.vector.tensor_mul(O_sb_p[:, j, hi, :], OPu[:, 0:64],
                                     Ri.broadcast_to([128, 64]))

            def transpose_j(qc, j, o_sbs):
                """Bring both pairs' (q-major) outputs for q-tile j back to
                channel-major: OS_pj [128 ch, 128 q] per pair."""
                os_j = []
                for p in range(2):
                    OTP = smp.tile([128, 128], BF16, name="OTP", tag="sm",
                                   space="PSUM")
                    nc.tensor.transpose(
                        OTP[:, :],
                        o_sbs[p][:, j, :, :], ident_sb[:, :])
                    OS = osp.tile([128, 128], BF16, name="OS", tag="os")
                    nc.vector.tensor_copy(OS[:], OTP[:])
                    os_j.append(OS)
                return os_j

            def outproj_j(qc, j, os_j, last_qc=False):
                ysb = ysp.tile([128, 2, 512], BF16, name="ysb", tag="ys")
                if last_qc:
                    # tail-latency path: both kp0 matmuls first (they only
                    # need pair 0's transpose), per-nch DMA on separate
                    # issue queues
                    yps = [smp.tile([128, 512], F32, name="yp", tag="sm")
                           for _ in range(2)]
                    for kp in range(2):
                        for nch in range(2):
                            nc.tensor.matmul(
                                yps[nch][:], os_j[kp][:, :],
                                wo_sb[:, kp, nch * 512:(nch + 1) * 512],
                                start=(kp == 0), stop=(kp == 1))
                    for nch in range(2):
                        eng = nc.vector if nch == 0 else nc.scalar
                        if nch == 0:
                            eng.tensor_copy(ysb[:, nch, :], yps[nch][:])
                        else:
                            eng.copy(ysb[:, nch, :], yps[nch][:])
                        dq = nc.sync if nch == 0 else nc.scalar
                        dq.dma_start(
                            y_r[:, qc * 4 + j, nch:nch + 1, :],
                            ysb[:, nch:nch + 1, :])
                    return
                for nch in range(2):
                    yp = smp.tile([128, 512], F32, name="yp", tag="sm")
                    for kp in range(2):
                        nc.tensor.matmul(
                            yp[:], os_j[kp][:, :],
                            wo_sb[:, kp, nch * 512:(nch + 1) * 512],
                            start=(kp == 0), stop=(kp == 1))
                    nc.vector.tensor_copy(ysb[:, nch, :], yp[:])
                nc.sync.dma_start(y_r[:, qc * 4 + j, :, :], ysb[:, :, :])

            # ---- main schedule ------------------------------------------
            # chunk 0 in two 256-col halves so the first matmul only waits
            # on wq + a quarter-chunk of x
            for half in range(2):
                for p in range(2):
                    for which in ("q", "k"):
                        for klo in (0, 4):
                            proj_qk(0, p, which, klo, half=half)
                for st in (0 + 2 * half, 1 + 2 * half):
                    for klo in (0, 4):
                        proj_v(st, klo)

            push_proj_chunks()
            bridges = [None, None]
            for qc in range(NQC):
                op_state["qc"] = qc
                drain_proj(qc)
                fill_late.extend(late_kv.pop(qc, []))
                last = qc == NQC - 1
                if qc == 2:
                    op_state["cap"] += OP_CAP2
                elif last:
                    op_state["cap"] = float("inf")
                ktmax = 4 * (qc + 1)
                ptws = [ptp.tile([128, NKT, 2, 512], BF16, name=f"PTW{p}",
                                 tag=f"ptw{p}") for p in range(2)]
                o_sbs = [onp.tile([128, 4, 2, 64], BF16, name=f"O_sb{p}",
                                  tag=f"ob{p}") for p in range(2)]
                for kt in range(ktmax):
                    if kt == 4 * qc:
                        # diagonal K tiles / V tiles of this chunk are
                        # about to be consumed
                        drain_late(qc)
                    for p in range(2):
                        if kt < BR_KT and bridges[p] is not None:
                            continue    # emitted during qc-1's j-phase
                        scores(qc, p, kt, ptws[p])
                    emit_filler(KT_BUDGET)
                cur_bridges = bridges
                bridges = [None, None]

                def finish_j(j):
                    os_j = transpose_j(qc, j, o_sbs)
                    if last:
                        outproj_j(qc, j, os_j, last_qc=True)
                    else:
                        fill_op.append((OP_EST, (
                            lambda qc=qc, j=j, o=os_j: outproj_j(qc, j, o))))

                # j-groups software-pipelined by one: bursts(j+1) run on PE
                # while DVE normalizes group j, so the transpose of group j
                # never stalls PE
                for j in range(4):
                    if last and j == 3:
                        # nothing may trail the final j-chain
                        drain_all()
                    for p in range(2):
                        for hi in range(2):
                            pv_burst(qc, p, hi, j, ptws[p], o_sbs[p],
                                     cur_bridges[p])
                        if j == 3 and not last:
                            # pair p's P-window is consumed: bridge the Act
                            # boundary by streaming the next q-chunk's
                            # first scores for this pair now
                            br = brp.tile([128, BR_KT, 2, 512], BF16,
                                          name="BR", tag=f"br{p}")
                            for kt in range(BR_KT):
                                scores(qc + 1, p, kt, br)
                            bridges[p] = br
                    emit_filler(J_BUDGET)
                    if j > 0:
                        finish_j(j - 1)
                finish_j(3)
            drain_all()

    if split_waits:
        _split_excess_waits(nc, max_waits=1)
    return nc


_NC = None


def _in_maps(x, Wq, Wk, Wv, Wo):
    bf = ml_dtypes.bfloat16
    x = np.asarray(x, dtype=np.float32)
    Wq, Wk, Wv, Wo = (np.asarray(w, dtype=np.float32) for w in (Wq, Wk, Wv, Wo))
    tri = np.triu(np.ones((128, 128), dtype=np.float32))  # m[k,q] = k<=q
    mask = np.ascontiguousarray(np.stack([tri, tri], axis=1)).astype(bf)
    ident = np.eye(128, dtype=np.float32).astype(bf)
    in_maps = []
    for core in range(8):
        b, g = divmod(core, G)
        csl = slice(g * CL, (g + 1) * CL)
        in_maps.append({
            "xt": np.ascontiguousarray(x[b].T).astype(bf),
            "wq": np.ascontiguousarray(Wq[csl, :].T).astype(bf),
            "wk": np.ascontiguousarray(Wk[csl, :].T).astype(bf),
            "wv": np.ascontiguousarray(Wv[csl, :].T).astype(bf),
            "wo": np.ascontiguousarray(
                Wo[:, csl].T.reshape(2, 128, D).transpose(1, 0, 2)).astype(bf),
            "mask": mask,
            "ident": ident,
        })
    return in_maps


def _sim_inputs(inputs):
    return _in_maps(**inputs)[0]


def kernel(x, Wq, Wk, Wv, Wo):
    global _NC
    if _NC is None:
        _NC = _build_nc()
    in_maps = _in_maps(x, Wq, Wk, Wv, Wo)
    res = run_bass_kernel_spmd(_NC, in_maps, list(range(8)))
    y = np.empty((B, S, D), dtype=np.float32)
    for b in range(B):
        acc = res.results[4 * b]["y"].astype(np.float32)
        for g in range(1, G):
            acc = acc + res.results[4 * b + g]["y"].astype(np.float32)
        y[b] = acc
    return y
